# revision 3
# baseline (speedup 1.0000x reference)
"""nn_BlockLinear Trainium2 kernel (8 NeuronCores, data-parallel over tokens).

Reference computation (per token t):
  xb = x.reshape(B, T, 16, 8, 16)                       # [c, m, k] feature blocks
  y[b,t,o,m,n] = sum_{c,k} xb[b,t,c,m,k] * w[o,c,n,k] + bias[o,m,n]
  out = y.reshape(B, T, 2048)

For each m this is the SAME 256x256 matmul applied to x_m[(c,k)] giving
y_m[(o,n)] — so per (token, m) pair: one 256-deep contraction.

Strategy:
  * Shard tokens (B*T = 16384) evenly over 8 cores; weight replicated.
    The shard copy also permutes features (c,m,k) -> (m,c,k) so every
    on-device transpose input is a contiguous 128-column block.
  * Per core, per 256-token macro tile:
      - DMA in [128p, 2, 2048] fp32 (2 MB, token-major, full DMA efficiency)
      - TensorE transposes (float32r): [128 tok, 128 (c',k)] -> [(c',k), tok] PSUM
      - ScalarE copies PSUM -> SBUF (xT)
      - TensorE matmuls (float32r, full rate at N=256): lhsT = xT chunk
        [(c,k)=128, tok=128], rhs = W chunk [(c,k)=128, (o,n)=256] -> PSUM
        [tok, (o,n)] fp32, accumulating the two 128-row c-halves
      - VectorE copies PSUM -> SBUF permuting (m,o,n) -> (o,m,n)
      - DMA out [128p, 2, 2048] fp32 (2 MB, natural output layout)
  * Bias is added on host only if nonzero (it is structurally zero here).

float32r (TF32-like reduced-precision fp32 matmul mode) measures ~1.5e-4
absmax relative error end-to-end — ~20x better than a bf16 pipeline —
while streaming at full PE rate.  The kernel is memory-roofline-bound:
~95 us of DMA at the ~358 GB/s per-core HBM limit out of ~107 us total.
"""

import sys

for _p in ("/opt/trn_rl_repo",):
    if _p not in sys.path:
        sys.path.append(_p)

import numpy as np

N_CORES = 8
C, M, K, O, N = 16, 8, 16, 8, 32
FIN = 2048
FOUT = 2048

_CACHE = {}


def _build(tok_per_core):
    import concourse.bacc as bacc
    import concourse.mybir as mybir
    from concourse import tile

    F32R = mybir.dt.float32r
    F32 = mybir.dt.float32
    nmacro = tok_per_core // 256

    nc = bacc.Bacc("TRN2", target_bir_lowering=False, debug=False,
                   num_devices=N_CORES)
    x_d = nc.dram_tensor("x", [tok_per_core, FIN], F32R, kind="ExternalInput")
    w_d = nc.dram_tensor("w", [2, 128, 256], F32R, kind="ExternalInput")
    i_d = nc.dram_tensor("ident", [128, 128], F32R, kind="ExternalInput")
    y_d = nc.dram_tensor("y", [tok_per_core, FOUT], F32, kind="ExternalOutput")

    with tile.TileContext(nc) as tc:
        with (
            tc.tile_pool(name="const", bufs=1) as cpool,
            tc.tile_pool(name="xin", bufs=3) as xpool,
            tc.tile_pool(name="xT", bufs=2) as xTpool,
            tc.tile_pool(name="yout", bufs=3) as ypool,
            tc.tile_pool(name="tp_ps", bufs=2, space="PSUM") as tppool,
            tc.tile_pool(name="y_ps", bufs=2, space="PSUM") as yppool,
        ):
            wt = cpool.tile([128, 2, 256], F32R)
            nc.sync.dma_start(wt[:], w_d[:].rearrange("c p n -> p c n"))
            idt = cpool.tile([128, 128], F32R)
            nc.sync.dma_start(idt[:], i_d[:])

            for i in range(nmacro):
                xt = xpool.tile([128, 2, FIN], F32R)
                nc.sync.dma_start(
                    xt[:],
                    x_d[i * 256:(i + 1) * 256, :].rearrange(
                        "(j p) f -> p j f", p=128),
                )

                # X^T: per sub-tile j, 16 chunks [(c',k)=128, 128 tok];
                # chunk q = m*2 + h (h = c-half) at xT[:, j, q*128:(q+1)*128]
                xT = xTpool.tile([128, 2, 16 * 128], F32R)
                for half in range(4):      # 8 transposes per 2-bank PSUM tile
                    tp = tppool.tile([128, 1024], F32R)
                    for s in range(8):
                        g = half * 8 + s   # g = j*16 + q
                        j, q = divmod(g, 16)
                        nc.tensor.transpose(
                            tp[:, s * 128:(s + 1) * 128],
                            xt[:, j, q * 128:(q + 1) * 128],
                            idt[:],
                        )
                    nc.scalar.copy(
                        xT[:].rearrange("p j f -> p (j f)")[
                            :, half * 1024:(half + 1) * 1024],
                        tp[:],
                    )

                yt = ypool.tile([128, 2, FOUT], F32)
                yt_r = yt[:].rearrange("p j (o m n) -> p j m o n",
                                       o=O, m=M, n=N)
                for half in range(4):      # 4 m's per 2-bank PSUM tile
                    yp = yppool.tile([128, 1024], F32)
                    j, u = divmod(half, 2)
                    for s in range(4):
                        m = u * 4 + s
                        out_sl = yp[:, s * 256:(s + 1) * 256]
                        nc.tensor.matmul(
                            out_sl,
                            xT[:, j, (2 * m) * 128:(2 * m + 1) * 128],
                            wt[:, 0], start=True, stop=False,
                        )
                        nc.tensor.matmul(
                            out_sl,
                            xT[:, j, (2 * m + 1) * 128:(2 * m + 2) * 128],
                            wt[:, 1], start=False, stop=True,
                        )
                    nc.vector.tensor_copy(
                        yt_r[:, j, u * 4:(u + 1) * 4, :, :],
                        yp[:].rearrange("p (m o n) -> p m o n", m=4, o=O, n=N),
                    )
                nc.scalar.dma_start(
                    y_d[i * 256:(i + 1) * 256, :].rearrange(
                        "(j p) f -> p j f", p=128),
                    yt[:],
                )

    nc.compile()
    return nc


def _prep_inputs(x, weight, per):
    """Shard + permute features (c,m,k) -> (m,c,k); build per-core in_maps."""
    ntok = x.shape[0] * x.shape[1]
    xs4 = x.reshape(ntok, C, M, K)
    # W'[(c,k),(o,n)] = weight[o,c,n,k], split into the two 128-row c-halves.
    wp = np.ascontiguousarray(weight.transpose(1, 3, 0, 2).reshape(256, 256))
    w2 = np.ascontiguousarray(np.stack([wp[:128], wp[128:]]))
    ident = np.eye(128, dtype=np.float32)
    return [
        {
            "x": np.ascontiguousarray(
                xs4[c * per:(c + 1) * per].transpose(0, 2, 1, 3)
            ).reshape(per, FIN),
            "w": w2,
            "ident": ident,
        }
        for c in range(N_CORES)
    ]


def kernel(x, weight, bias, **run_kwargs):
    """Full inputs in, full output out.  Shards over 8 NeuronCores inside."""
    from concourse.bass_utils import run_bass_kernel_spmd

    x = np.asarray(x, dtype=np.float32)
    weight = np.asarray(weight, dtype=np.float32)
    bias = np.asarray(bias, dtype=np.float32)
    Bdim, Tdim, _ = x.shape
    ntok = Bdim * Tdim
    per = ntok // N_CORES
    assert per % 256 == 0, f"tokens per core ({per}) must be a multiple of 256"

    if per not in _CACHE:
        _CACHE[per] = _build(per)
    nc = _CACHE[per]

    in_maps = _prep_inputs(x, weight, per)
    res = run_bass_kernel_spmd(nc, in_maps, core_ids=list(range(N_CORES)),
                               **run_kwargs)
    kernel.last_result = res  # for local profiling harnesses
    y = np.concatenate([r["y"] for r in res.results], axis=0)
    y = y.reshape(Bdim, Tdim, FOUT)
    if np.any(bias):
        y = (y.reshape(Bdim, Tdim, O, M, N) + bias).reshape(Bdim, Tdim, FOUT)
    return y.astype(np.float32, copy=False)


# revision 4
# speedup vs baseline: 1.0940x; 1.0940x over previous
"""nn_BlockLinear Trainium2 kernel (8 NeuronCores, data-parallel over tokens).

Reference computation (per token t):
  xb = x.reshape(B, T, 16, 8, 16)                       # [c, m, k] feature blocks
  y[b,t,o,m,n] = sum_{c,k} xb[b,t,c,m,k] * w[o,c,n,k] + bias[o,m,n]
  out = y.reshape(B, T, 2048)

For each m this is the SAME 256x256 matmul applied to x_m[(c,k)] giving
y_m[(o,n)] — so per (token, m) pair: one 256-deep contraction.

Strategy:
  * Shard tokens (B*T = 16384) evenly over 8 cores; weight replicated.
    The shard copy also permutes features (c,m,k) -> (m,c,k) so every
    on-device transpose input is a contiguous 128-column block.
  * Per core, per 256-token macro tile:
      - DMA in [128p, 2, 2048] fp32 (2 MB, token-major, full DMA efficiency)
      - TensorE transposes (float32r): [128 tok, 128 (c',k)] -> [(c',k), tok] PSUM
      - ScalarE copies PSUM -> SBUF (xT)
      - TensorE matmuls (float32r, full rate at N=256): lhsT = xT chunk
        [(c,k)=128, tok=128], rhs = W chunk [(c,k)=128, (o,n)=256] -> PSUM
        [tok, (o,n)] fp32, accumulating the two 128-row c-halves
      - VectorE copies PSUM -> SBUF permuting (m,o,n) -> (o,m,n)
      - DMA out [128p, 2, 2048] fp32 (2 MB, natural output layout)
  * Bias is added on host only if nonzero (it is structurally zero here).

float32r (TF32-like reduced-precision fp32 matmul mode) measures ~1.5e-4
absmax relative error end-to-end — ~20x better than a bf16 pipeline —
while streaming at full PE rate.  The kernel is memory-roofline-bound:
~95 us of DMA at the ~358 GB/s per-core HBM limit out of ~107 us total.
"""

import sys

for _p in ("/opt/trn_rl_repo",):
    if _p not in sys.path:
        sys.path.append(_p)

import numpy as np

N_CORES = 8
C, M, K, O, N = 16, 8, 16, 8, 32
FIN = 2048
FOUT = 2048

_CACHE = {}


def _build(tok_per_core):
    import concourse.bacc as bacc
    import concourse.mybir as mybir
    from concourse import tile

    F32R = mybir.dt.float32r
    F32 = mybir.dt.float32
    nmacro = tok_per_core // 256

    nc = bacc.Bacc("TRN2", target_bir_lowering=False, debug=False,
                   num_devices=N_CORES)
    x_d = nc.dram_tensor("x", [tok_per_core, FIN], F32R, kind="ExternalInput")
    w_d = nc.dram_tensor("w", [2, 128, 256], F32R, kind="ExternalInput")
    i_d = nc.dram_tensor("ident", [128, 128], F32R, kind="ExternalInput")
    y_d = nc.dram_tensor("y", [tok_per_core, FOUT], F32, kind="ExternalOutput")

    with tile.TileContext(nc) as tc:
        with (
            tc.tile_pool(name="const", bufs=1) as cpool,
            tc.tile_pool(name="xin", bufs=4) as xpool,
            tc.tile_pool(name="xT", bufs=2) as xTpool,
            tc.tile_pool(name="yout", bufs=4) as ypool,
            tc.tile_pool(name="tp_ps", bufs=2, space="PSUM") as tppool,
            tc.tile_pool(name="y_ps", bufs=2, space="PSUM") as yppool,
        ):
            wt = cpool.tile([128, 2, 256], F32R)
            nc.sync.dma_start(wt[:], w_d[:].rearrange("c p n -> p c n"))
            idt = cpool.tile([128, 128], F32R)
            nc.sync.dma_start(idt[:], i_d[:])

            for i in range(nmacro):
                xt = xpool.tile([128, 2, FIN], F32R)
                nc.sync.dma_start(
                    xt[:],
                    x_d[i * 256:(i + 1) * 256, :].rearrange(
                        "(j p) f -> p j f", p=128),
                )

                # X^T: per sub-tile j, 16 chunks [(c',k)=128, 128 tok];
                # chunk q = m*2 + h (h = c-half) at xT[:, j, q*128:(q+1)*128]
                xT = xTpool.tile([128, 2, 16 * 128], F32R)
                for half in range(4):      # 8 transposes per 2-bank PSUM tile
                    tp = tppool.tile([128, 1024], F32R)
                    for s in range(8):
                        g = half * 8 + s   # g = j*16 + q
                        j, q = divmod(g, 16)
                        nc.tensor.transpose(
                            tp[:, s * 128:(s + 1) * 128],
                            xt[:, j, q * 128:(q + 1) * 128],
                            idt[:],
                        )
                    nc.scalar.copy(
                        xT[:].rearrange("p j f -> p (j f)")[
                            :, half * 1024:(half + 1) * 1024],
                        tp[:],
                    )

                yt = ypool.tile([128, 2, FOUT], F32)
                yt_r = yt[:].rearrange("p j (o m n) -> p j m o n",
                                       o=O, m=M, n=N)
                for half in range(4):      # 4 m's per 2-bank PSUM tile
                    yp = yppool.tile([128, 1024], F32)
                    j, u = divmod(half, 2)
                    for s in range(4):
                        m = u * 4 + s
                        out_sl = yp[:, s * 256:(s + 1) * 256]
                        nc.tensor.matmul(
                            out_sl,
                            xT[:, j, (2 * m) * 128:(2 * m + 1) * 128],
                            wt[:, 0], start=True, stop=False,
                        )
                        nc.tensor.matmul(
                            out_sl,
                            xT[:, j, (2 * m + 1) * 128:(2 * m + 2) * 128],
                            wt[:, 1], start=False, stop=True,
                        )
                    nc.vector.tensor_copy(
                        yt_r[:, j, u * 4:(u + 1) * 4, :, :],
                        yp[:].rearrange("p (m o n) -> p m o n", m=4, o=O, n=N),
                    )
                nc.scalar.dma_start(
                    y_d[i * 256:(i + 1) * 256, :].rearrange(
                        "(j p) f -> p j f", p=128),
                    yt[:],
                )

    nc.compile()
    return nc


def _prep_inputs(x, weight, per):
    """Shard + permute features (c,m,k) -> (m,c,k); build per-core in_maps."""
    ntok = x.shape[0] * x.shape[1]
    xs4 = x.reshape(ntok, C, M, K)
    # W'[(c,k),(o,n)] = weight[o,c,n,k], split into the two 128-row c-halves.
    wp = np.ascontiguousarray(weight.transpose(1, 3, 0, 2).reshape(256, 256))
    w2 = np.ascontiguousarray(np.stack([wp[:128], wp[128:]]))
    ident = np.eye(128, dtype=np.float32)
    return [
        {
            "x": np.ascontiguousarray(
                xs4[c * per:(c + 1) * per].transpose(0, 2, 1, 3)
            ).reshape(per, FIN),
            "w": w2,
            "ident": ident,
        }
        for c in range(N_CORES)
    ]


def kernel(x, weight, bias, **run_kwargs):
    """Full inputs in, full output out.  Shards over 8 NeuronCores inside."""
    from concourse.bass_utils import run_bass_kernel_spmd

    x = np.asarray(x, dtype=np.float32)
    weight = np.asarray(weight, dtype=np.float32)
    bias = np.asarray(bias, dtype=np.float32)
    Bdim, Tdim, _ = x.shape
    ntok = Bdim * Tdim
    per = ntok // N_CORES
    assert per % 256 == 0, f"tokens per core ({per}) must be a multiple of 256"

    if per not in _CACHE:
        _CACHE[per] = _build(per)
    nc = _CACHE[per]

    in_maps = _prep_inputs(x, weight, per)
    res = run_bass_kernel_spmd(nc, in_maps, core_ids=list(range(N_CORES)),
                               **run_kwargs)
    kernel.last_result = res  # for local profiling harnesses
    y = np.concatenate([r["y"] for r in res.results], axis=0)
    y = y.reshape(Bdim, Tdim, FOUT)
    if np.any(bias):
        y = (y.reshape(Bdim, Tdim, O, M, N) + bias).reshape(Bdim, Tdim, FOUT)
    return y.astype(np.float32, copy=False)


# revision 6
# speedup vs baseline: 1.0984x; 1.0039x over previous
"""nn_BlockLinear Trainium2 kernel (8 NeuronCores, data-parallel over tokens).

Reference computation (per token t):
  xb = x.reshape(B, T, 16, 8, 16)                       # [c, m, k] feature blocks
  y[b,t,o,m,n] = sum_{c,k} xb[b,t,c,m,k] * w[o,c,n,k] + bias[o,m,n]
  out = y.reshape(B, T, 2048)

For each m this is the SAME 256x256 matmul applied to x_m[(c,k)] giving
y_m[(o,n)] — so per (token, m) pair: one 256-deep contraction.

Strategy:
  * Shard tokens (B*T = 16384) evenly over 8 cores; weight replicated.
    The shard copy also permutes features (c,m,k) -> (m,c,k) so every
    on-device transpose input is a contiguous 128-column block.
  * Per core, per 256-token macro tile:
      - DMA in [128p, 2, 2048] fp32 (2 MB, token-major, full DMA efficiency)
      - TensorE transposes (float32r): [128 tok, 128 (c',k)] -> [(c',k), tok] PSUM
      - ScalarE copies PSUM -> SBUF (xT)
      - TensorE matmuls (float32r, full rate at N=256): lhsT = xT chunk
        [(c,k)=128, tok=128], rhs = W chunk [(c,k)=128, (o,n)=256] -> PSUM
        [tok, (o,n)] fp32, accumulating the two 128-row c-halves
      - VectorE copies PSUM -> SBUF permuting (m,o,n) -> (o,m,n)
      - DMA out [128p, 2, 2048] fp32 (2 MB, natural output layout)
  * Bias is added on host only if nonzero (it is structurally zero here).

float32r (TF32-like reduced-precision fp32 matmul mode) measures ~1.5e-4
absmax relative error end-to-end — ~20x better than a bf16 pipeline —
while streaming at full PE rate.  The kernel is memory-roofline-bound:
~95 us of DMA at the ~358 GB/s per-core HBM limit out of ~107 us total.
"""

import sys

for _p in ("/opt/trn_rl_repo",):
    if _p not in sys.path:
        sys.path.append(_p)

import numpy as np

N_CORES = 8
C, M, K, O, N = 16, 8, 16, 8, 32
FIN = 2048
FOUT = 2048

_CACHE = {}


def _build(tok_per_core):
    import concourse.bacc as bacc
    import concourse.mybir as mybir
    from concourse import tile

    F32R = mybir.dt.float32r
    F32 = mybir.dt.float32
    nmacro = tok_per_core // 256

    nc = bacc.Bacc("TRN2", target_bir_lowering=False, debug=False,
                   num_devices=N_CORES)
    x_d = nc.dram_tensor("x", [tok_per_core, FIN], F32R, kind="ExternalInput")
    w_d = nc.dram_tensor("w", [2, 128, 256], F32R, kind="ExternalInput")
    i_d = nc.dram_tensor("ident", [128, 128], F32R, kind="ExternalInput")
    y_d = nc.dram_tensor("y", [tok_per_core, FOUT], F32, kind="ExternalOutput")

    with tile.TileContext(nc) as tc:
        with (
            tc.tile_pool(name="const", bufs=1) as cpool,
            tc.tile_pool(name="xin", bufs=4) as xpool,
            tc.tile_pool(name="xT", bufs=2) as xTpool,
            tc.tile_pool(name="yout", bufs=4) as ypool,
            tc.tile_pool(name="tp_ps", bufs=2, space="PSUM") as tppool,
            tc.tile_pool(name="y_ps", bufs=2, space="PSUM") as yppool,
        ):
            wt = cpool.tile([128, 2, 256], F32R)
            idt = cpool.tile([128, 128], F32R)

            for i in range(nmacro):
                xt = xpool.tile([128, 2, FIN], F32R)
                if i == nmacro - 1:
                    # split the last tile's input per 128-token half so the
                    # tail compute chain starts on half 0 sooner
                    for j in range(2):
                        nc.sync.dma_start(
                            xt[:, j],
                            x_d[(2 * i + j) * 128:(2 * i + j + 1) * 128, :])
                else:
                    nc.sync.dma_start(
                        xt[:],
                        x_d[i * 256:(i + 1) * 256, :].rearrange(
                            "(j p) f -> p j f", p=128),
                    )
                if i == 0:
                    # consts issued after x0 so the 2 MB x0 leads the SP ring
                    nc.sync.dma_start(idt[:], i_d[:])
                    nc.sync.dma_start(wt[:], w_d[:].rearrange("c p n -> p c n"))

                # X^T: per sub-tile j, 16 chunks [(c',k)=128, 128 tok];
                # chunk q = m*2 + h (h = c-half) at xT[:, j, q*128:(q+1)*128]
                xT = xTpool.tile([128, 2, 16 * 128], F32R)
                for half in range(4):      # 8 transposes per 2-bank PSUM tile
                    tp = tppool.tile([128, 1024], F32R)
                    for s in range(8):
                        g = half * 8 + s   # g = j*16 + q
                        j, q = divmod(g, 16)
                        nc.tensor.transpose(
                            tp[:, s * 128:(s + 1) * 128],
                            xt[:, j, q * 128:(q + 1) * 128],
                            idt[:],
                        )
                    nc.scalar.copy(
                        xT[:].rearrange("p j f -> p (j f)")[
                            :, half * 1024:(half + 1) * 1024],
                        tp[:],
                    )

                yt = ypool.tile([128, 2, FOUT], F32)
                yt_r = yt[:].rearrange("p j (o m n) -> p j m o n",
                                       o=O, m=M, n=N)
                for half in range(4):      # 4 m's per 2-bank PSUM tile
                    yp = yppool.tile([128, 1024], F32)
                    j, u = divmod(half, 2)
                    for s in range(4):
                        m = u * 4 + s
                        out_sl = yp[:, s * 256:(s + 1) * 256]
                        nc.tensor.matmul(
                            out_sl,
                            xT[:, j, (2 * m) * 128:(2 * m + 1) * 128],
                            wt[:, 0], start=True, stop=False,
                        )
                        nc.tensor.matmul(
                            out_sl,
                            xT[:, j, (2 * m + 1) * 128:(2 * m + 2) * 128],
                            wt[:, 1], start=False, stop=True,
                        )
                    nc.vector.tensor_copy(
                        yt_r[:, j, u * 4:(u + 1) * 4, :, :],
                        yp[:].rearrange("p (m o n) -> p m o n", m=4, o=O, n=N),
                    )
                if i == nmacro - 1:
                    # split the last tile's output per half: y(j=0) streams
                    # while half 1 is still computing
                    for j in range(2):
                        nc.scalar.dma_start(
                            y_d[(2 * i + j) * 128:(2 * i + j + 1) * 128, :],
                            yt[:, j],
                        )
                else:
                    nc.scalar.dma_start(
                        y_d[i * 256:(i + 1) * 256, :].rearrange(
                            "(j p) f -> p j f", p=128),
                        yt[:],
                    )

    nc.compile()
    return nc


def _prep_inputs(x, weight, per):
    """Shard + permute features (c,m,k) -> (m,c,k); build per-core in_maps."""
    ntok = x.shape[0] * x.shape[1]
    xs4 = x.reshape(ntok, C, M, K)
    # W'[(c,k),(o,n)] = weight[o,c,n,k], split into the two 128-row c-halves.
    wp = np.ascontiguousarray(weight.transpose(1, 3, 0, 2).reshape(256, 256))
    w2 = np.ascontiguousarray(np.stack([wp[:128], wp[128:]]))
    ident = np.eye(128, dtype=np.float32)
    return [
        {
            "x": np.ascontiguousarray(
                xs4[c * per:(c + 1) * per].transpose(0, 2, 1, 3)
            ).reshape(per, FIN),
            "w": w2,
            "ident": ident,
        }
        for c in range(N_CORES)
    ]


def kernel(x, weight, bias, **run_kwargs):
    """Full inputs in, full output out.  Shards over 8 NeuronCores inside."""
    from concourse.bass_utils import run_bass_kernel_spmd

    x = np.asarray(x, dtype=np.float32)
    weight = np.asarray(weight, dtype=np.float32)
    bias = np.asarray(bias, dtype=np.float32)
    Bdim, Tdim, _ = x.shape
    ntok = Bdim * Tdim
    per = ntok // N_CORES
    assert per % 256 == 0, f"tokens per core ({per}) must be a multiple of 256"

    if per not in _CACHE:
        _CACHE[per] = _build(per)
    nc = _CACHE[per]

    in_maps = _prep_inputs(x, weight, per)
    res = run_bass_kernel_spmd(nc, in_maps, core_ids=list(range(N_CORES)),
                               **run_kwargs)
    kernel.last_result = res  # for local profiling harnesses
    y = np.concatenate([r["y"] for r in res.results], axis=0)
    y = y.reshape(Bdim, Tdim, FOUT)
    if np.any(bias):
        y = (y.reshape(Bdim, Tdim, O, M, N) + bias).reshape(Bdim, Tdim, FOUT)
    return y.astype(np.float32, copy=False)


# revision 7
# speedup vs baseline: 1.3326x; 1.2133x over previous
"""nn_BlockLinear Trainium2 kernel (8 NeuronCores, data-parallel over tokens).

Reference computation (per token t):
  xb = x.reshape(B, T, 16, 8, 16)                       # [c, m, k] feature blocks
  y[b,t,o,m,n] = sum_{c,k} xb[b,t,c,m,k] * w[o,c,n,k] + bias[o,m,n]
  out = y.reshape(B, T, 2048)

For each m this is the SAME 256x256 matmul applied to x_m[(c,k)] giving
y_m[(o,n)] — so per (token, m) pair: one 256-deep contraction.

Strategy:
  * Shard tokens (B*T = 16384) evenly over 8 cores; weight replicated.
    The shard copy permutes features (c,m,k) -> (m,c,k) so every on-device
    transpose input is a contiguous 128-column block, and casts x to fp16
    (measured absmax rel err 3.2e-4 vs the fp32 reference — the randn
    activations and +-0.011-bounded weights fit fp16's 10-bit mantissa; an
    order of magnitude more accurate than bf16).  fp16 halves the input
    HBM traffic: 25.3 MB/core total vs 33.9 MB at fp32.
  * Per core, per 256-token macro tile, per 128-token half j (transposes(j)
    then matmuls(j) interleaved so real matmuls land inside every PE-HAM
    activity window and the PE stays at 2.4 GHz):
      - DMA in [128p, 2, 2048] fp16 (1 MB, token-major)
      - TensorE transposes: [128 tok, 128 (c',k)] -> [(c',k), tok] PSUM
      - ScalarE copies PSUM -> SBUF (xT)
      - TensorE matmuls: lhsT = xT chunk [(c,k)=128, tok=128], rhs = W chunk
        [(c,k)=128, (o,n)=256] -> PSUM [tok, (o,n)] fp32, accumulating the
        two 128-row c-halves
      - VectorE copies PSUM -> SBUF fp32, permuting (m,o,n) -> (o,m,n)
      - DMA out [128p, 2, 2048] fp32 (2 MB, natural output layout)
  * The LAST macro tile's input/output DMAs are split per 128-token half so
    the final compute chain is short (removes the end-of-stream DMA idle).
  * Bias is added on host only if nonzero (it is structurally zero here).

Memory-roofline-bound: ~70 us of DMA at the ~358 GB/s per-core HBM limit out
of ~82 us total (plus ~2.7 us HBM startup latency and a fixed ~6.5 us
runtime epilogue).  PE 50 us, ScalarE 39 us, VectorE 40 us all overlap
underneath the DMA stream.
"""

import sys

for _p in ("/opt/trn_rl_repo",):
    if _p not in sys.path:
        sys.path.append(_p)

import numpy as np

N_CORES = 8
C, M, K, O, N = 16, 8, 16, 8, 32
FIN = 2048
FOUT = 2048

_CACHE = {}


def _build(tok_per_core):
    import concourse.bacc as bacc
    import concourse.mybir as mybir
    from concourse import tile

    F16 = mybir.dt.float16
    F32 = mybir.dt.float32
    nmacro = tok_per_core // 256

    nc = bacc.Bacc("TRN2", target_bir_lowering=False, debug=False,
                   num_devices=N_CORES)
    x_d = nc.dram_tensor("x", [tok_per_core, FIN], F16, kind="ExternalInput")
    w_d = nc.dram_tensor("w", [2, 128, 256], F16, kind="ExternalInput")
    i_d = nc.dram_tensor("ident", [128, 128], F16, kind="ExternalInput")
    y_d = nc.dram_tensor("y", [tok_per_core, FOUT], F32, kind="ExternalOutput")

    with tile.TileContext(nc) as tc:
        with (
            tc.tile_pool(name="const", bufs=1) as cpool,
            tc.tile_pool(name="xin", bufs=4) as xpool,
            tc.tile_pool(name="xT", bufs=2) as xTpool,
            tc.tile_pool(name="yout", bufs=4) as ypool,
            tc.tile_pool(name="tp_ps", bufs=2, space="PSUM") as tppool,
            tc.tile_pool(name="y_ps", bufs=3, space="PSUM") as yppool,
        ):
            wt = cpool.tile([128, 2, 256], F16)
            idt = cpool.tile([128, 128], F16)

            for i in range(nmacro):
                xt = xpool.tile([128, 2, FIN], F16)
                if i == nmacro - 1:
                    # split the last tile's input per 128-token half so the
                    # tail compute chain starts on half 0 sooner
                    for j in range(2):
                        nc.sync.dma_start(
                            xt[:, j],
                            x_d[(2 * i + j) * 128:(2 * i + j + 1) * 128, :])
                else:
                    nc.sync.dma_start(
                        xt[:],
                        x_d[i * 256:(i + 1) * 256, :].rearrange(
                            "(j p) f -> p j f", p=128),
                    )
                if i == 0:
                    # consts issued after x0 so the 1 MB x0 leads the SP ring
                    nc.sync.dma_start(idt[:], i_d[:])
                    nc.sync.dma_start(wt[:], w_d[:].rearrange("c p n -> p c n"))

                # Per 128-token half j: transposes(j) then matmuls(j), so
                # real MMs land inside every HAM window (PE stays warm).
                # xT chunk q = m*2 + h (h = c-half) at xT[:, j, q*128:...]
                xT = xTpool.tile([128, 2, 16 * 128], F16)
                yt = ypool.tile([128, 2, FOUT], F32)
                yt_r = yt[:].rearrange("p j (o m n) -> p j m o n",
                                       o=O, m=M, n=N)
                for j in range(2):
                    for half in range(2):   # 8 transposes per 1-bank PSUM tile
                        tp = tppool.tile([128, 1024], F16)
                        for s in range(8):
                            q = half * 8 + s
                            nc.tensor.transpose(
                                tp[:, s * 128:(s + 1) * 128],
                                xt[:, j, q * 128:(q + 1) * 128],
                                idt[:],
                            )
                        nc.scalar.copy(
                            xT[:, j, half * 1024:(half + 1) * 1024],
                            tp[:],
                        )
                    for u in range(2):      # 4 m's per 2-bank PSUM tile
                        yp = yppool.tile([128, 1024], F32)
                        for s in range(4):
                            m = u * 4 + s
                            out_sl = yp[:, s * 256:(s + 1) * 256]
                            nc.tensor.matmul(
                                out_sl,
                                xT[:, j, (2 * m) * 128:(2 * m + 1) * 128],
                                wt[:, 0], start=True, stop=False,
                            )
                            nc.tensor.matmul(
                                out_sl,
                                xT[:, j, (2 * m + 1) * 128:(2 * m + 2) * 128],
                                wt[:, 1], start=False, stop=True,
                            )
                        nc.vector.tensor_copy(
                            yt_r[:, j, u * 4:(u + 1) * 4, :, :],
                            yp[:].rearrange("p (m o n) -> p m o n",
                                            m=4, o=O, n=N),
                        )

                if i == nmacro - 1:
                    # split the last tile's output per half: y(j=0) streams
                    # while half 1 is still computing
                    for j in range(2):
                        nc.scalar.dma_start(
                            y_d[(2 * i + j) * 128:(2 * i + j + 1) * 128, :],
                            yt[:, j],
                        )
                else:
                    nc.scalar.dma_start(
                        y_d[i * 256:(i + 1) * 256, :].rearrange(
                            "(j p) f -> p j f", p=128),
                        yt[:],
                    )

    nc.compile()
    return nc


def _prep_inputs(x, weight, per):
    """Shard + permute features (c,m,k) -> (m,c,k), cast to fp16."""
    ntok = x.shape[0] * x.shape[1]
    xs4 = x.reshape(ntok, C, M, K)
    # W'[(c,k),(o,n)] = weight[o,c,n,k], split into the two 128-row c-halves.
    wp = np.ascontiguousarray(weight.transpose(1, 3, 0, 2).reshape(256, 256))
    w2 = np.stack([wp[:128], wp[128:]]).astype(np.float16)
    ident = np.eye(128, dtype=np.float16)
    return [
        {
            "x": np.ascontiguousarray(
                xs4[c * per:(c + 1) * per].transpose(0, 2, 1, 3)
            ).reshape(per, FIN).astype(np.float16),
            "w": w2,
            "ident": ident,
        }
        for c in range(N_CORES)
    ]


def kernel(x, weight, bias, **run_kwargs):
    """Full inputs in, full output out.  Shards over 8 NeuronCores inside."""
    from concourse.bass_utils import run_bass_kernel_spmd

    x = np.asarray(x, dtype=np.float32)
    weight = np.asarray(weight, dtype=np.float32)
    bias = np.asarray(bias, dtype=np.float32)
    Bdim, Tdim, _ = x.shape
    ntok = Bdim * Tdim
    per = ntok // N_CORES
    assert per % 256 == 0, f"tokens per core ({per}) must be a multiple of 256"

    if per not in _CACHE:
        _CACHE[per] = _build(per)
    nc = _CACHE[per]

    in_maps = _prep_inputs(x, weight, per)
    res = run_bass_kernel_spmd(nc, in_maps, core_ids=list(range(N_CORES)),
                               **run_kwargs)
    kernel.last_result = res  # for local profiling harnesses
    y = np.concatenate([r["y"] for r in res.results], axis=0)
    y = y.reshape(Bdim, Tdim, FOUT)
    if np.any(bias):
        y = (y.reshape(Bdim, Tdim, O, M, N) + bias).reshape(Bdim, Tdim, FOUT)
    return y.astype(np.float32, copy=False)


# revision 8
# speedup vs baseline: 1.6680x; 1.2516x over previous
"""nn_BlockLinear Trainium2 kernel (8 NeuronCores, data-parallel over tokens).

Reference computation (per token t):
  xb = x.reshape(B, T, 16, 8, 16)                       # [c, m, k] feature blocks
  y[b,t,o,m,n] = sum_{c,k} xb[b,t,c,m,k] * w[o,c,n,k] + bias[o,m,n]
  out = y.reshape(B, T, 2048)

For each m this is the SAME 256x256 matmul applied to x_m[(c,k)] giving
y_m[(o,n)] — so per (token, m) pair: one 256-deep contraction.

Strategy:
  * Shard tokens (B*T = 16384) evenly over 8 cores; weight replicated.
    The shard copy permutes features (c,m,k) -> (m,c,k) so every on-device
    transpose input is a contiguous 128-column block, and casts x to fp16
    (measured absmax rel err 3.2e-4 vs the fp32 reference — the randn
    activations and +-0.011-bounded weights fit fp16's 10-bit mantissa; an
    order of magnitude more accurate than bf16).  fp16 halves the input
    HBM traffic: 25.3 MB/core total vs 33.9 MB at fp32.
  * Per core, per 256-token macro tile, per 128-token half j (transposes(j)
    then matmuls(j) interleaved so real matmuls land inside every PE-HAM
    activity window and the PE stays at 2.4 GHz):
      - DMA in [128p, 2, 2048] fp16 (1 MB, token-major)
      - TensorE transposes: [128 tok, 128 (c',k)] -> [(c',k), tok] PSUM
      - ScalarE copies PSUM -> SBUF (xT)
      - TensorE matmuls: lhsT = xT chunk [(c,k)=128, tok=128], rhs = W chunk
        [(c,k)=128, (o,n)=256] -> PSUM [tok, (o,n)] fp32, accumulating the
        two 128-row c-halves
      - VectorE copies PSUM -> SBUF fp32, permuting (m,o,n) -> (o,m,n)
      - DMA out [128p, 2, 2048] fp16 (1 MB, natural layout; host upcasts)
  * The LAST macro tile's input/output DMAs are split per 128-token half so
    the final compute chain is short (removes the end-of-stream DMA idle).
  * Bias is added on host only if nonzero (it is structurally zero here).

Memory-roofline-bound: ~70 us of DMA at the ~358 GB/s per-core HBM limit out
of ~82 us total (plus ~2.7 us HBM startup latency and a fixed ~6.5 us
runtime epilogue).  PE 50 us, ScalarE 39 us, VectorE 40 us all overlap
underneath the DMA stream.
"""

import sys

for _p in ("/opt/trn_rl_repo",):
    if _p not in sys.path:
        sys.path.append(_p)

import numpy as np

N_CORES = 8
C, M, K, O, N = 16, 8, 16, 8, 32
FIN = 2048
FOUT = 2048

_CACHE = {}


def _build(tok_per_core):
    import concourse.bacc as bacc
    import concourse.mybir as mybir
    from concourse import tile

    F16 = mybir.dt.float16
    F32 = mybir.dt.float32
    nmacro = tok_per_core // 256

    nc = bacc.Bacc("TRN2", target_bir_lowering=False, debug=False,
                   num_devices=N_CORES)
    x_d = nc.dram_tensor("x", [tok_per_core, FIN], F16, kind="ExternalInput")
    w_d = nc.dram_tensor("w", [2, 128, 256], F16, kind="ExternalInput")
    i_d = nc.dram_tensor("ident", [128, 128], F16, kind="ExternalInput")
    y_d = nc.dram_tensor("y", [tok_per_core, FOUT], F16, kind="ExternalOutput")

    with tile.TileContext(nc) as tc:
        with (
            tc.tile_pool(name="const", bufs=1) as cpool,
            tc.tile_pool(name="xin", bufs=4) as xpool,
            tc.tile_pool(name="xT", bufs=2) as xTpool,
            tc.tile_pool(name="yout", bufs=4) as ypool,
            tc.tile_pool(name="tp_ps", bufs=3, space="PSUM") as tppool,
            tc.tile_pool(name="y_ps", bufs=5, space="PSUM") as yppool,
        ):
            wt = cpool.tile([128, 2, 256], F16)
            idt = cpool.tile([128, 128], F16)

            for i in range(nmacro):
                xt = xpool.tile([128, 2, FIN], F16)
                if i == nmacro - 1:
                    # split the last tile's input per 128-token half so the
                    # tail compute chain starts on half 0 sooner
                    for j in range(2):
                        nc.sync.dma_start(
                            xt[:, j],
                            x_d[(2 * i + j) * 128:(2 * i + j + 1) * 128, :])
                else:
                    nc.sync.dma_start(
                        xt[:],
                        x_d[i * 256:(i + 1) * 256, :].rearrange(
                            "(j p) f -> p j f", p=128),
                    )
                if i == 0:
                    # consts issued after x0 so the 1 MB x0 leads the SP ring
                    nc.sync.dma_start(idt[:], i_d[:])
                    nc.sync.dma_start(wt[:], w_d[:].rearrange("c p n -> p c n"))

                # Per 128-token half j: transposes(j) then matmuls(j), so
                # real MMs land inside every HAM window (PE stays warm).
                # xT chunk q = m*2 + h (h = c-half) at xT[:, j, q*128:...]
                xT = xTpool.tile([128, 2, 16 * 128], F16)
                yt = ypool.tile([128, 2, FOUT], F16)
                yt_r = yt[:].rearrange("p j (o m n) -> p j m o n",
                                       o=O, m=M, n=N)
                for j in range(2):
                    for half in range(2):   # 8 transposes per 1-bank PSUM tile
                        tp = tppool.tile([128, 1024], F16)
                        for s in range(8):
                            q = half * 8 + s
                            nc.tensor.transpose(
                                tp[:, s * 128:(s + 1) * 128],
                                xt[:, j, q * 128:(q + 1) * 128],
                                idt[:],
                            )
                        nc.scalar.copy(
                            xT[:, j, half * 1024:(half + 1) * 1024],
                            tp[:],
                        )
                    for u in range(4):      # 2 m's per 1-bank PSUM tile
                        yp = yppool.tile([128, 512], F32)
                        for s in range(2):
                            m = u * 2 + s
                            out_sl = yp[:, s * 256:(s + 1) * 256]
                            nc.tensor.matmul(
                                out_sl,
                                xT[:, j, (2 * m) * 128:(2 * m + 1) * 128],
                                wt[:, 0], start=True, stop=False,
                            )
                            nc.tensor.matmul(
                                out_sl,
                                xT[:, j, (2 * m + 1) * 128:(2 * m + 2) * 128],
                                wt[:, 1], start=False, stop=True,
                            )
                        nc.vector.tensor_copy(
                            yt_r[:, j, u * 2:(u + 1) * 2, :, :],
                            yp[:].rearrange("p (m o n) -> p m o n",
                                            m=2, o=O, n=N),
                        )

                if i == nmacro - 1:
                    # split the last tile's output per half: y(j=0) streams
                    # while half 1 is still computing
                    for j in range(2):
                        nc.scalar.dma_start(
                            y_d[(2 * i + j) * 128:(2 * i + j + 1) * 128, :],
                            yt[:, j],
                        )
                else:
                    nc.scalar.dma_start(
                        y_d[i * 256:(i + 1) * 256, :].rearrange(
                            "(j p) f -> p j f", p=128),
                        yt[:],
                    )

    nc.compile()
    return nc


def _prep_inputs(x, weight, per):
    """Shard + permute features (c,m,k) -> (m,c,k), cast to fp16."""
    ntok = x.shape[0] * x.shape[1]
    xs4 = x.reshape(ntok, C, M, K)
    # W'[(c,k),(o,n)] = weight[o,c,n,k], split into the two 128-row c-halves.
    wp = np.ascontiguousarray(weight.transpose(1, 3, 0, 2).reshape(256, 256))
    w2 = np.stack([wp[:128], wp[128:]]).astype(np.float16)
    ident = np.eye(128, dtype=np.float16)
    return [
        {
            "x": np.ascontiguousarray(
                xs4[c * per:(c + 1) * per].transpose(0, 2, 1, 3)
            ).reshape(per, FIN).astype(np.float16),
            "w": w2,
            "ident": ident,
        }
        for c in range(N_CORES)
    ]


def kernel(x, weight, bias, **run_kwargs):
    """Full inputs in, full output out.  Shards over 8 NeuronCores inside."""
    from concourse.bass_utils import run_bass_kernel_spmd

    x = np.asarray(x, dtype=np.float32)
    weight = np.asarray(weight, dtype=np.float32)
    bias = np.asarray(bias, dtype=np.float32)
    Bdim, Tdim, _ = x.shape
    ntok = Bdim * Tdim
    per = ntok // N_CORES
    assert per % 256 == 0, f"tokens per core ({per}) must be a multiple of 256"

    if per not in _CACHE:
        _CACHE[per] = _build(per)
    nc = _CACHE[per]

    in_maps = _prep_inputs(x, weight, per)
    res = run_bass_kernel_spmd(nc, in_maps, core_ids=list(range(N_CORES)),
                               **run_kwargs)
    kernel.last_result = res  # for local profiling harnesses
    y = np.concatenate([r["y"].astype(np.float32) for r in res.results],
                       axis=0)
    y = y.reshape(Bdim, Tdim, FOUT)
    if np.any(bias):
        y = (y.reshape(Bdim, Tdim, O, M, N) + bias).reshape(Bdim, Tdim, FOUT)
    return y.astype(np.float32, copy=False)
